# revision 1
# baseline (speedup 1.0000x reference)
"""Trainium2 Bass kernel for nn_DGL_GCN (3-layer hetero GCN + MLP head).

Math (reference): 3x hetero layers
    h' = relu( sum_e segment_mean_e( h @ W_e.T + b_e ) )
then z = relu(fc1_w @ h3.flatten() + fc1_b); out = sigmoid(fc2_w @ z + fc2_b).

The per-etype mean aggregation over edges is algebraically
    A_e @ (h @ W_e.T) + ind_e (x) b_e
with A_e[dst, src] = multiplicity(src->dst) / max(deg(dst),1) and
ind_e[dst] = deg(dst) > 0. A_e is a fixed 4096x4096 matrix per etype, so
each layer is dense matmuls on the PE array (far faster than per-edge
gather/scatter DMA on this hardware).

Sharding over 8 cores: destination-node shards (512 dst nodes per core,
all 8 etypes on-core -> all cross-etype sums happen in fp32 PSUM, no
AllReduce needed; one small AllGather of h per layer). fc1 is column-
sharded over the flattened node*hidden dim (each core's own h3 shard is
exactly its fc1 column slice, so no gather before fc1); partial z vectors
are AllReduce-summed.

All matmuls run in bf16 with fp32 PSUM accumulation.
"""

import numpy as np
import ml_dtypes

N_OBJ = 4096
F_IN = 256
H = 256
C = 128
NE = 8
NCORES = 8
SHARD = N_OBJ // NCORES          # 512 dst nodes per core
NT = N_OBJ // 128                # 32 node tiles
KB = 8                           # src k-tiles batched per A DMA
FCB = 16                         # fc1 k-tiles batched per DMA
FCK = (SHARD * H) // 128         # 1024 fc1 k-tiles per core

BF16 = ml_dtypes.bfloat16
FP8 = ml_dtypes.float8_e4m3
FC1_SCALE = 8192.0  # fc1_w ~N(0, 0.002) is subnormal in e4m3; pre-scale
H_SCALE = 16.0      # hidden state h ~0.05-0.3 also sits near e4m3 subnormals;
                    # keep the whole h-stream in S*h domain (relu commutes)

_BASS_CACHE = {}


def _split_drain_waits(nc, max_waits=1):
    # This walrus build accepts only one sync-wait command on an InstDrain;
    # Tile's tail drain waits on every active proc lane. Split into a chain
    # of single-wait drains.
    import copy
    import concourse.mybir as mybir

    for f in nc.m.functions:
        for bb in f.blocks:
            new_list = []
            for ins in bb.instructions:
                si = ins.sync_info
                if (
                    isinstance(ins, mybir.InstDrain)
                    and si is not None
                    and si.on_wait
                    and len(si.on_wait) > max_waits
                ):
                    waits = list(si.on_wait)
                    updates = list(si.on_update or [])
                    for i, w in enumerate(waits[:-1]):
                        d = copy.deepcopy(ins)
                        d.name = f"{ins.name}-sw{i}"
                        dsi = d.sync_info
                        dsi.on_wait = [w]
                        dsi.on_update = []
                        d.sync_info = dsi
                        new_list.append(d)
                        nc.inst_map[d.name] = d
                    si.on_wait = [waits[-1]]
                    si.on_update = updates
                    ins.sync_info = si
                new_list.append(ins)
            bb.instructions[:] = new_list


def _build_bass(n_layers=3, with_fc=True):
    import concourse.bass as bass  # noqa: F401
    import concourse.tile as tile
    import concourse.mybir as mybir
    from concourse import bacc

    f32 = mybir.dt.float32
    bf16 = mybir.dt.bfloat16
    AF = mybir.ActivationFunctionType

    nc = bacc.Bacc(
        "TRN2", target_bir_lowering=False, debug=False, num_devices=NCORES
    )

    # ---- I/O (per-core values supplied via in_maps) ----
    fp8 = mybir.dt.float8e4
    G0 = nc.dram_tensor("g0", [2, 128, N_OBJ], fp8, kind="ExternalInput")
    ATP = nc.dram_tensor("atp", [NE, NT // KB, 128, KB * SHARD], bf16, kind="ExternalInput")
    WT = nc.dram_tensor("wt", [128, 3 * 2 * NE, H], bf16, kind="ExternalInput")
    BIA = nc.dram_tensor("bia", [1, 3 * NE, H], bf16, kind="ExternalInput")
    IND = nc.dram_tensor("ind", [1, NE, SHARD], bf16, kind="ExternalInput")
    FC1T = nc.dram_tensor("fc1t", [FCK // FCB, 128, FCB * H], fp8, kind="ExternalInput")
    FC1B = nc.dram_tensor("fc1b", [128, 2], f32, kind="ExternalInput")
    FC2T = nc.dram_tensor("fc2t", [128, 2 * C], bf16, kind="ExternalInput")
    FC2B = nc.dram_tensor("fc2b", [128, 1], f32, kind="ExternalInput")
    OUT = nc.dram_tensor("out", [C, 1], f32, kind="ExternalOutput")

    rg = [list(range(NCORES))]

    with tile.TileContext(nc) as tc:
        with (
            tc.tile_pool(name="wpool", bufs=1) as wpool,
            tc.tile_pool(name="gpool", bufs=2) as gpool,
            tc.tile_pool(name="xpool", bufs=2) as xpool,
            tc.tile_pool(name="atpool", bufs=4) as atpool,
            tc.tile_pool(name="fcpool", bufs=10) as fcpool,
            tc.tile_pool(name="spool", bufs=2) as spool,
            tc.tile_pool(name="pxp", bufs=2, space="PSUM") as pxp,
            tc.tile_pool(name="pgp", bufs=1, space="PSUM") as pgp,
            tc.tile_pool(name="pzp", bufs=1, space="PSUM") as pzp,
            tc.tile_pool(name="dram", bufs=2, space="DRAM") as dram,
        ):
            # ---- initial G = feat.T first: layer-0 critical path ----
            g = []
            for k in range(2):
                gt = gpool.tile([128, N_OBJ], fp8, tag=f"g{k}", name=f"g_l0_{k}")
                nc.sync.dma_start(gt[:], G0[k])
                g.append(gt)

            # ---- resident small weights ----
            wt_sb = wpool.tile([128, 3 * 2 * NE, H], bf16)
            nc.sync.dma_start(wt_sb[:], WT[:])
            bia_sb = wpool.tile([1, 3 * NE, H], bf16)
            nc.sync.dma_start(bia_sb[:], BIA[:])
            ind_sb = wpool.tile([1, NE, SHARD], bf16)
            nc.sync.dma_start(ind_sb[:], IND[:])
            fc1b_sb = wpool.tile([128, 2], f32)
            nc.sync.dma_start(fc1b_sb[:], FC1B[:])
            fc2t_sb = wpool.tile([128, 2 * C], bf16)
            nc.sync.dma_start(fc2t_sb[:], FC2T[:])
            fc2b_sb = wpool.tile([128, 1], f32)
            nc.sync.dma_start(fc2b_sb[:], FC2B[:])

            # warmup collective, shape-identical to the layer AllGather: pays
            # the one-time ncfw/collective init hidden under layer-1 compute
            wuin = dram.tile([2, 128, SHARD], fp8, tag="wuin")
            for m in range(2):
                nc.gpsimd.dma_start(wuin[m], ATP[0, 0][:, 0:SHARD])
            wuout = dram.tile(
                [NCORES, 2, 128, SHARD], fp8, tag="wuout", addr_space="Shared"
            )
            nc.gpsimd.collective_compute(
                "AllGather",
                mybir.AluOpType.bypass,
                replica_groups=rg,
                ins=[wuin.opt()],
                outs=[wuout.opt()],
            )

            g3sh = None
            for layer in range(n_layers):
                # layer-long PSUM accumulators for out.T = [H, dst_shard]
                pg = [
                    pgp.tile([128, SHARD], f32, tag=f"pg{m}", name=f"pg_l{layer}_{m}")
                    for m in range(2)
                ]
                for pr in range(NE // 2):
                    # Wh for an etype PAIR in one N=512 matmul stream:
                    # x_sb[:, mt, sub*H:(sub+1)*H] = X_{2pr+sub} node-tile mt
                    x_sb = xpool.tile(
                        [128, NT, 2 * H], bf16, tag="x", name=f"x_l{layer}_p{pr}"
                    )
                    for mt in range(NT):
                        px = pxp.tile([128, 512], f32, tag="px")
                        for k in range(2):
                            wslice = wt_sb[
                                :, (layer * 2 + k) * NE + 2 * pr : (layer * 2 + k) * NE + 2 * pr + 2, :
                            ].rearrange("p a b -> p (a b)")
                            nc.tensor.matmul(
                                px[:],
                                lhsT=g[k][:, mt * 128 : mt * 128 + 128],
                                rhs=wslice,
                                start=(k == 0),
                                stop=(k == 1),
                            )
                        if mt % 2 == 0:
                            nc.vector.tensor_copy(x_sb[:, mt, :], px[:])
                        else:
                            nc.scalar.copy(x_sb[:, mt, :], px[:])
                    for sub in range(2):
                        e = 2 * pr + sub
                        # A-multiply: pg[m] += X_e[k].T-slices @ AT_e[k]
                        for k8 in range(NT // KB):
                            at8 = atpool.tile([128, KB, SHARD], bf16, tag="at")
                            nc.sync.dma_start(
                                at8[:], ATP[e, k8].rearrange("p (s j) -> p s j", s=KB)
                            )
                            for s in range(KB):
                                kt = k8 * KB + s
                                for m in range(2):
                                    nc.tensor.matmul(
                                        pg[m][:],
                                        lhsT=x_sb[
                                            :, kt, sub * H + m * 128 : sub * H + m * 128 + 128
                                        ],
                                        rhs=at8[:, s, :],
                                        start=(e == 0 and kt == 0),
                                        stop=False,
                                    )
                        # bias: pg[m] += b_e[h-slice] (x) ind_e[dst]
                        for m in range(2):
                            nc.tensor.matmul(
                                pg[m][:],
                                lhsT=bia_sb[:, layer * NE + e, m * 128 : m * 128 + 128],
                                rhs=ind_sb[:, e, :],
                                start=False,
                                stop=(e == NE - 1),
                            )
                if layer < n_layers - 1:
                    # relu -> fp8 shard (stay in H_SCALE domain; relu commutes)
                    gsh = []
                    for m in range(2):
                        gs = spool.tile(
                            [128, SHARD], fp8, tag=f"gshq{m}", name=f"gshq_l{layer}_{m}"
                        )
                        nc.scalar.activation(gs[:], pg[m][:], AF.Relu)
                        gsh.append(gs)
                    agin = dram.tile([2, 128, SHARD], fp8, tag="agin")
                    for m in range(2):
                        nc.gpsimd.dma_start(agin[m], gsh[m][:])
                    agout = dram.tile(
                        [NCORES, 2, 128, SHARD], fp8, tag="agout", addr_space="Shared"
                    )
                    nc.gpsimd.collective_compute(
                        "AllGather",
                        mybir.AluOpType.bypass,
                        replica_groups=rg,
                        ins=[agin.opt()],
                        outs=[agout.opt()],
                    )
                    g = []
                    for k in range(2):
                        gt = gpool.tile(
                            [128, N_OBJ], fp8, tag=f"g{k}", name=f"g_l{layer + 1}_{k}"
                        )
                        nc.sync.dma_start(
                            gt[:].rearrange("p (c j) -> p c j", c=NCORES),
                            agout[:, k, :, :].rearrange("c p j -> p c j"),
                        )
                        g.append(gt)
                else:
                    g3sh = []
                    for m in range(2):
                        gs = spool.tile(
                            [128, SHARD], bf16, tag=f"gsh{m}", name=f"gsh_l{layer}_{m}"
                        )
                        nc.scalar.activation(gs[:], pg[m][:], AF.Relu)
                        g3sh.append(gs)

            if not with_fc:
                osb0 = spool.tile([C, 1], f32, tag="osb")
                nc.vector.tensor_copy(osb0[:], g3sh[0][:, 0:1])
                nc.gpsimd.dma_start(OUT[:], osb0[:])
            else:
                # ---- fc1: z_partial[1, 256] = sum_t flat_t.T @ fc1T_t ----
                pz = pzp.tile([1, H], f32, tag="pz")
                for blk in range(FCK // FCB):
                    w16 = fcpool.tile([128, FCB, H], fp8, tag="fc1")
                    nc.sync.dma_start(
                        w16[:], FC1T[blk].rearrange("p (s f) -> p s f", s=FCB)
                    )
                    for s in range(FCB):
                        t = blk * FCB + s
                        nc.tensor.matmul(
                            pz[:],
                            lhsT=g3sh[t % 2][:, t // 2 : t // 2 + 1],
                            rhs=w16[:, s, :],
                            start=(t == 0),
                            stop=(t == FCK - 1),
                        )
                zsb = spool.tile([1, H], f32, tag="zsb")
                nc.vector.tensor_copy(zsb[:], pz[:])

                # AllGather per-core z partials, then sum over ranks on the PE
                # with a K=8 ones-matmul — which also transposes z into the
                # [128, 1] column layout fc2 needs.
                agzin = dram.tile([1, H], f32, tag="agzin")
                nc.gpsimd.dma_start(agzin[:], zsb[:])
                agzout = dram.tile([NCORES, 1, H], f32, tag="agzout", addr_space="Shared")
                nc.gpsimd.collective_compute(
                    "AllGather",
                    mybir.AluOpType.bypass,
                    replica_groups=rg,
                    ins=[agzin.opt()],
                    outs=[agzout.opt()],
                )
                zparts = spool.tile([NCORES, H], f32, tag="zparts")
                nc.sync.dma_start(zparts[:], agzout[:, 0, :])
                ones8 = wpool.tile([NCORES, 1], f32)
                nc.gpsimd.memset(ones8[:], 1.0)

                po = pzp.tile([C, 1], f32, tag="po")
                for k in range(2):
                    poz = pzp.tile([128, 1], f32, tag=f"poz{k}")
                    nc.tensor.matmul(
                        poz[:],
                        lhsT=zparts[:, k * 128 : (k + 1) * 128],
                        rhs=ones8[:],
                        start=True,
                        stop=True,
                    )
                    zr = spool.tile([128, 1], bf16, tag=f"zr{k}")
                    nc.scalar.activation(
                        zr[:],
                        poz[:],
                        AF.Relu,
                        bias=fc1b_sb[:, k : k + 1],
                        scale=1.0 / (FC1_SCALE * H_SCALE),
                    )
                    nc.tensor.matmul(
                        po[:],
                        lhsT=fc2t_sb[:, k * C : (k + 1) * C],
                        rhs=zr[:],
                        start=(k == 0),
                        stop=(k == 1),
                    )
                osb = spool.tile([C, 1], f32, tag="osb")
                nc.scalar.activation(osb[:], po[:], AF.Sigmoid, bias=fc2b_sb[:, 0:1])
                nc.gpsimd.dma_start(OUT[:], osb[:])

    nc.compile()
    _split_drain_waits(nc)
    return nc


def _prep_shared(feat, W0, b0, W1, b1, W2, b2, fc1_b, fc2_w, fc2_b):
    """Host layout prep for the tensors every core receives identically."""
    g0 = (
        (np.ascontiguousarray(feat.T) * H_SCALE).astype(FP8).reshape(2, 128, N_OBJ)
    )

    # index order (layer, k, e) so an etype PAIR is contiguous: the Wh
    # matmuls run two etypes per MM with a [128, 512] rhs slice
    wt = np.empty((128, 3 * 2 * NE, H), dtype=BF16)
    for li, W in enumerate((W0, W1, W2)):
        for e in range(NE):
            wte = np.ascontiguousarray(W[e].T).astype(BF16)  # [F, H]
            wt[:, (li * 2 + 0) * NE + e, :] = wte[:128]
            wt[:, (li * 2 + 1) * NE + e, :] = wte[128:]

    bia = np.empty((1, 3 * NE, H), dtype=BF16)
    for li, b in enumerate((b0, b1, b2)):
        bia[0, li * NE : (li + 1) * NE, :] = (b * H_SCALE).astype(BF16)

    fc1b = np.ascontiguousarray(fc1_b.reshape(2, 128).T).astype(np.float32)
    fc2t = np.ascontiguousarray(
        fc2_w.T.reshape(2, 128, C).transpose(1, 0, 2).reshape(128, 2 * C)
    ).astype(BF16)
    fc2b = fc2_b.reshape(C, 1).astype(np.float32)
    return g0, wt, bia, fc1b, fc2t, fc2b


def _prep_graph(edges):
    """Per-(etype, core) normalized adjacency slices + degree indicators."""
    atp = np.empty((NCORES, NE, NT // KB, 128, KB * SHARD), dtype=BF16)
    ind = np.empty((NCORES, 1, NE, SHARD), dtype=BF16)
    for e in range(NE):
        src = np.asarray(edges[e, 0], dtype=np.int64)
        dst = np.asarray(edges[e, 1], dtype=np.int64)
        deg = np.bincount(dst, minlength=N_OBJ)
        cnt = np.bincount(src * N_OBJ + dst, minlength=N_OBJ * N_OBJ)
        an = (
            cnt.reshape(N_OBJ, N_OBJ).astype(np.float32)
            / np.maximum(deg, 1).astype(np.float32)[None, :]
        )
        ind_e = (deg > 0).astype(np.float32)
        for c in range(NCORES):
            sh = an[:, c * SHARD : (c + 1) * SHARD]  # [4096 src, 512]
            atp[c, e] = (
                sh.reshape(NT // KB, KB, 128, SHARD)
                .transpose(0, 2, 1, 3)
                .reshape(NT // KB, 128, KB * SHARD)
                .astype(BF16)
            )
            ind[c, 0, e] = ind_e[c * SHARD : (c + 1) * SHARD].astype(BF16)
    return atp, ind


def _prep_fc1(fc1_w):
    """Per-core column slice of fc1_w, transposed and DMA-batch packed."""
    out = []
    ksl = SHARD * H  # 131072 flat positions per core
    for c in range(NCORES):
        sl = np.ascontiguousarray(fc1_w[:, c * ksl : (c + 1) * ksl].T)  # [131072, 256]
        packed = (
            (sl.reshape(FCK // FCB, FCB, 128, H) * FC1_SCALE)
            .transpose(0, 2, 1, 3)
            .reshape(FCK // FCB, 128, FCB * H)
            .astype(FP8)
        )
        out.append(packed)
    return out


def kernel(feat, edges, W0, b0, W1, b1, W2, b2, fc1_w, fc1_b, fc2_w, fc2_b):
    from concourse.bass_utils import run_bass_kernel_spmd

    if "nc" not in _BASS_CACHE:
        _BASS_CACHE["nc"] = _build_bass()
    nc = _BASS_CACHE["nc"]

    in_maps = _make_in_maps(
        dict(
            feat=feat, edges=edges, W0=W0, b0=b0, W1=W1, b1=b1, W2=W2, b2=b2,
            fc1_w=fc1_w, fc1_b=fc1_b, fc2_w=fc2_w, fc2_b=fc2_b,
        )
    )

    res = run_bass_kernel_spmd(nc, in_maps, core_ids=list(range(NCORES)))
    out = np.asarray(res.results[0]["out"]).reshape(C)
    return out.astype(np.float32)


def _make_in_maps(inputs):
    g0, wt, bia, fc1b, fc2t, fc2b = _prep_shared(
        np.asarray(inputs["feat"], dtype=np.float32),
        np.asarray(inputs["W0"]), np.asarray(inputs["b0"]),
        np.asarray(inputs["W1"]), np.asarray(inputs["b1"]),
        np.asarray(inputs["W2"]), np.asarray(inputs["b2"]),
        np.asarray(inputs["fc1_b"]), np.asarray(inputs["fc2_w"]),
        np.asarray(inputs["fc2_b"]),
    )
    atp, ind = _prep_graph(np.asarray(inputs["edges"]))
    fc1t = _prep_fc1(np.asarray(inputs["fc1_w"]))
    return [
        {
            "g0": g0, "atp": atp[c], "wt": wt, "bia": bia, "ind": ind[c],
            "fc1t": fc1t[c], "fc1b": fc1b, "fc2t": fc2t, "fc2b": fc2b,
        }
        for c in range(NCORES)
    ]


def run_profiled(inputs, trace_cores=None):
    """Test-only: run with NTFF tracing; returns BassKernelResults."""
    from concourse import bass_utils
    from concourse.bass_utils import run_bass_kernel_spmd

    bass_utils.upload_artifacts = lambda tmpdir: f"local://{tmpdir}"
    if "nc" not in _BASS_CACHE:
        _BASS_CACHE["nc"] = _build_bass()
    nc = _BASS_CACHE["nc"]
    in_maps = _make_in_maps(inputs)
    tmpdir = "/tmp/gcn_profile"
    import shutil, os
    shutil.rmtree(tmpdir, ignore_errors=True)
    os.makedirs(tmpdir, exist_ok=True)
    return run_bass_kernel_spmd(
        nc,
        in_maps,
        core_ids=list(range(NCORES)),
        trace=True,
        tmpdir=tmpdir,
        trace_cores=trace_cores,
    )



# revision 2
# speedup vs baseline: 1.7909x; 1.7909x over previous
"""Trainium2 Bass kernel for nn_DGL_GCN (3-layer hetero GCN + MLP head).

Math (reference): 3x hetero layers
    h' = relu( sum_e segment_mean_e( h @ W_e.T + b_e ) )
then z = relu(fc1_w @ h3.flatten() + fc1_b); out = sigmoid(fc2_w @ z + fc2_b).

Key algebra: A_e @ (h @ W_e.T) == (A_e @ h) @ W_e.T, so each core
aggregates first (contraction over all 4096 src for its own 512 dst)
and transforms only its 512-dst aggregate -- no redundant per-core Wh.

A_e[src, dst] = cnt(src->dst)/max(deg(dst),1) is stored fp8 as
cnt * q_d with q_d = fp8(A_SCALE/deg_d) (exact for cnt in {1,2,4});
the residual per-(etype,dst) scale (1/deg)/q is applied exactly at the
PSUM drain. A-multiply and fc1 run fp8 DoubleRow (2 fp8/cell, 2x rate).

Sharding: destination-node shards (512 dst/core, all 8 etypes on-core,
cross-etype sum in the transform PSUM). fc1 column-sharded over the
flattened node*hidden dim; own h3 shard == own fc1 column slice, partial
z AllGathered then rank-summed on the PE.
"""

import numpy as np
import ml_dtypes

N_OBJ = 4096
F_IN = 256
H = 256
C = 128
NE = 8
NCORES = 8
SHARD = N_OBJ // NCORES          # 512 dst nodes per core
NCH = 16                         # src chunks of 256 (DoubleRow K-tiles)
NCHP = 8                         # chunk pairs (A DMA batching)
R_RES = 2                        # etypes kept SBUF-resident across layers
FC1_NBLK = 32                    # fc1 blocks of 16 chunks (4096 flat k each)
FC1_BUFS = 8                     # fc1 prefetch ring depth (8 KB/partition each)

BF16 = ml_dtypes.bfloat16
FP8 = ml_dtypes.float8_e4m3
H_SCALE = 16.0    # hidden state kept in S*h domain (relu commutes)
FC1_SCALE = 8192.0
A_SCALE = 4.0     # folded into the drain correction

_BASS_CACHE = {}

PASSES = ((0, 1, 2), (3, 4, 5), (6, 7))


def _split_drain_waits(nc, max_waits=1):
    # This walrus build accepts only one sync-wait command on an InstDrain;
    # Tile's tail drain waits on every active proc lane. Split into a chain
    # of single-wait drains.
    import copy
    import concourse.mybir as mybir

    for f in nc.m.functions:
        for bb in f.blocks:
            new_list = []
            for ins in bb.instructions:
                si = ins.sync_info
                if (
                    isinstance(ins, mybir.InstDrain)
                    and si is not None
                    and si.on_wait
                    and len(si.on_wait) > max_waits
                ):
                    waits = list(si.on_wait)
                    updates = list(si.on_update or [])
                    for i, w in enumerate(waits[:-1]):
                        d = copy.deepcopy(ins)
                        d.name = f"{ins.name}-sw{i}"
                        dsi = d.sync_info
                        dsi.on_wait = [w]
                        dsi.on_update = []
                        d.sync_info = dsi
                        new_list.append(d)
                        nc.inst_map[d.name] = d
                    si.on_wait = [waits[-1]]
                    si.on_update = updates
                    ins.sync_info = si
                new_list.append(ins)
            bb.instructions[:] = new_list


def _build_bass(has_bias=False):
    import concourse.bass as bass  # noqa: F401
    import concourse.tile as tile
    import concourse.mybir as mybir
    from concourse import bacc

    f32 = mybir.dt.float32
    bf16 = mybir.dt.bfloat16
    fp8 = mybir.dt.float8e4
    AF = mybir.ActivationFunctionType
    DR = mybir.MatmulPerfMode.DoubleRow
    ALU = mybir.AluOpType

    nc = bacc.Bacc(
        "TRN2", target_bir_lowering=False, debug=False, num_devices=NCORES
    )

    # ---- I/O (per-core values supplied via in_maps) ----
    G0 = nc.dram_tensor("g0", [128, NCH, 2, F_IN], fp8, kind="ExternalInput")
    ATP = nc.dram_tensor(
        "atp", [NE, NCHP, 128, 2, 2, SHARD], fp8, kind="ExternalInput"
    )
    CORR = nc.dram_tensor("corr", [128, NE, SHARD], f32, kind="ExternalInput")
    WT = nc.dram_tensor("wt", [128, 3 * NE * 2, H], bf16, kind="ExternalInput")
    FC1T = nc.dram_tensor(
        "fc1t", [FC1_NBLK, 128, NCH, 2, H], fp8, kind="ExternalInput"
    )
    FC1B = nc.dram_tensor("fc1b", [128, 2], f32, kind="ExternalInput")
    FC2T = nc.dram_tensor("fc2t", [128, 2 * C], bf16, kind="ExternalInput")
    FC2B = nc.dram_tensor("fc2b", [128, 1], f32, kind="ExternalInput")
    if has_bias:
        HBN = nc.dram_tensor("hbn", [128, 2, 4, H], bf16, kind="ExternalInput")
        HBT = nc.dram_tensor("hbt", [128, 2, SHARD], bf16, kind="ExternalInput")
    OUT = nc.dram_tensor("out", [C, 1], f32, kind="ExternalOutput")

    rg = [list(range(NCORES))]

    with tile.TileContext(nc) as tc:
        with (
            tc.tile_pool(name="wpool", bufs=1) as wpool,
            tc.tile_pool(name="gpool", bufs=2) as gpool,
            tc.tile_pool(name="arespool", bufs=1) as arespool,
            tc.tile_pool(name="atpool", bufs=6) as atpool,
            tc.tile_pool(name="aggpool", bufs=16) as aggpool,
            tc.tile_pool(name="hpool", bufs=2) as hpool,
            tc.tile_pool(name="fcpool", bufs=FC1_BUFS) as fcpool,
            tc.tile_pool(name="spool", bufs=2) as spool,
            tc.tile_pool(name="aggp", bufs=6, space="PSUM") as aggp,
            tc.tile_pool(name="hxp", bufs=2, space="PSUM") as hxp,
            tc.tile_pool(name="dram", bufs=2, space="DRAM") as dram,
        ):
            # ---- initial g (feat, DoubleRow-interleaved natural layout) ----
            g = gpool.tile([128, NCH, 2, F_IN], fp8, tag="g", name="g_l0")
            nc.sync.dma_start(g[:], G0[:])

            # ---- resident A etypes (loaded once, reused all 3 layers) ----
            ares = None
            if R_RES > 0:
                ares = arespool.tile(
                    [128, R_RES, NCHP, 2, 2, SHARD], fp8, name="ares"
                )
                for e in range(R_RES):
                    for chp in range(NCHP):
                        nc.sync.dma_start(ares[:, e, chp], ATP[e, chp])

            # ---- resident small weights ----
            wt_sb = wpool.tile([128, 3 * NE * 2, H], bf16)
            nc.sync.dma_start(wt_sb[:], WT[:])
            corr_sb = wpool.tile([128, NE, SHARD], f32)
            nc.sync.dma_start(corr_sb[:], CORR[:])
            fc1b_sb = wpool.tile([128, 2], f32)
            nc.sync.dma_start(fc1b_sb[:], FC1B[:])
            fc2t_sb = wpool.tile([128, 2 * C], bf16)
            nc.sync.dma_start(fc2t_sb[:], FC2T[:])
            fc2b_sb = wpool.tile([128, 1], f32)
            nc.sync.dma_start(fc2b_sb[:], FC2B[:])
            ones8 = wpool.tile([NCORES, 1], f32)
            nc.gpsimd.memset(ones8[:], 1.0)
            if has_bias:
                hbn_sb = wpool.tile([128, 2, 4, H], bf16)
                nc.sync.dma_start(hbn_sb[:], HBN[:])
                hbt_sb = wpool.tile([128, 2, SHARD], bf16)
                nc.sync.dma_start(hbt_sb[:], HBT[:])

            # warmup collective, shape-identical to the layer AllGather: pays
            # the one-time ncfw/collective init hidden under layer-0 compute
            wuin = dram.tile([128, 4, F_IN], fp8, tag="agin", name="wuin")
            nc.gpsimd.dma_start(
                wuin[:], G0[:, 0:2, :, :].rearrange("p a i f -> p (a i) f")
            )
            wuout = dram.tile(
                [NCORES, 128, 4, F_IN], fp8, tag="agout", addr_space="Shared",
                name="wuout",
            )
            nc.gpsimd.collective_compute(
                "AllGather",
                ALU.bypass,
                replica_groups=rg,
                ins=[wuin.opt()],
                outs=[wuout.opt()],
            )

            fc1_tiles = []

            def issue_fc1(blk):
                t = fcpool.tile(
                    [128, NCH, 2, H], fp8, tag="fc1", name=f"fc1_{blk}"
                )
                nc.scalar.dma_start(t[:], FC1T[blk])
                fc1_tiles.append(t)

            h3q = None
            for layer in range(3):
                aggT = {}
                for pe in PASSES:
                    # per-etype aggregate PSUMs for this pass
                    pg = {
                        (e, fh): aggp.tile(
                            [128, SHARD], f32, tag="agg",
                            name=f"pg_l{layer}_e{e}_f{fh}",
                        )
                        for e in pe
                        for fh in range(2)
                    }
                    for chp in range(NCHP):
                        at_t = {}
                        for e in pe:
                            if e < R_RES:
                                at_t[e] = ares[:, e, chp]
                            else:
                                t = atpool.tile(
                                    [128, 2, 2, SHARD], fp8, tag="at",
                                    name=f"at_l{layer}_e{e}_c{chp}",
                                )
                                nc.sync.dma_start(t[:], ATP[e, chp])
                                at_t[e] = t
                        for sub in range(2):
                            ch = chp * 2 + sub
                            for fh in range(2):
                                lhsT = g[:, ch, :, fh * 128 : (fh + 1) * 128]
                                for e in pe:
                                    nc.tensor.matmul(
                                        pg[(e, fh)][:],
                                        lhsT=lhsT,
                                        rhs=at_t[e][:, sub],
                                        start=(ch == 0),
                                        stop=(ch == NCH - 1),
                                        perf_mode=DR,
                                    )
                    # drain with exact per-(etype,dst) fp8-A correction
                    for e in pe:
                        for fh in range(2):
                            t = aggpool.tile(
                                [128, SHARD], bf16, tag="aggT",
                                name=f"aggT_l{layer}_e{e}_f{fh}",
                            )
                            nc.vector.scalar_tensor_tensor(
                                t[:], pg[(e, fh)][:], 1.0, corr_sb[:, e, :],
                                ALU.bypass, ALU.mult,
                            )
                            aggT[(e, fh)] = t

                if layer < 2:
                    # transform: h_next[dst, :] = relu(sum_e agg_e @ W_e.T)
                    hsh = hpool.tile(
                        [128, 4, H], fp8, tag="hsh", name=f"hsh_l{layer}"
                    )
                    for dt in range(4):
                        ph = hxp.tile(
                            [128, H], f32, tag="hx", name=f"ph_l{layer}_d{dt}"
                        )
                        for e in range(NE):
                            for fh in range(2):
                                nc.tensor.matmul(
                                    ph[:],
                                    lhsT=aggT[(e, fh)][
                                        :, dt * 128 : (dt + 1) * 128
                                    ],
                                    rhs=wt_sb[:, (layer * NE + e) * 2 + fh, :],
                                    start=(e == 0 and fh == 0),
                                    stop=(e == NE - 1 and fh == 1),
                                )
                        if has_bias:
                            nc.vector.scalar_tensor_tensor(
                                ph[:], ph[:], 1.0, hbn_sb[:, layer, dt, :],
                                ALU.bypass, ALU.add,
                            )
                        nc.scalar.activation(hsh[:, dt, :], ph[:], AF.Relu)
                    agin = dram.tile(
                        [128, 4, H], fp8, tag="agin", name=f"agin_l{layer}"
                    )
                    nc.gpsimd.dma_start(agin[:], hsh[:])
                    agout = dram.tile(
                        [NCORES, 128, 4, H], fp8, tag="agout",
                        addr_space="Shared", name=f"agout_l{layer}",
                    )
                    nc.gpsimd.collective_compute(
                        "AllGather",
                        ALU.bypass,
                        replica_groups=rg,
                        ins=[agin.opt()],
                        outs=[agout.opt()],
                    )
                    g = gpool.tile(
                        [128, NCH, 2, F_IN], fp8, tag="g", name=f"g_l{layer+1}"
                    )
                    for c in range(NCORES):
                        nc.sync.dma_start(
                            g[:, 2 * c : 2 * c + 2, :, :],
                            agout[c].rearrange("p (a i) f -> p a i f", a=2),
                        )
                else:
                    # final layer: produce h3.T (fc1 lhsT layout), fp8
                    h3q = hpool.tile([128, 2, SHARD], fp8, name="h3q")
                    for jh in range(2):
                        ph3 = hxp.tile(
                            [128, SHARD], f32, tag="hx", name=f"ph3_j{jh}"
                        )
                        for e in range(NE):
                            for fh in range(2):
                                nc.tensor.matmul(
                                    ph3[:],
                                    lhsT=wt_sb[
                                        :, (layer * NE + e) * 2 + fh,
                                        jh * 128 : (jh + 1) * 128,
                                    ],
                                    rhs=aggT[(e, fh)][:],
                                    start=(e == 0 and fh == 0),
                                    stop=(e == NE - 1 and fh == 1),
                                )
                        if has_bias:
                            nc.vector.scalar_tensor_tensor(
                                ph3[:], ph3[:], 1.0, hbt_sb[:, jh, :],
                                ALU.bypass, ALU.add,
                            )
                        nc.scalar.activation(h3q[:, jh, :], ph3[:], AF.Relu)

                if layer == 0:
                    # fc1 prefetch: ring-depth blocks land during layers 1-2
                    for blk in range(FC1_BUFS):
                        issue_fc1(blk)

            # ---- fc1: z_partial[1, 256], fp8 DoubleRow GEMV ----
            pz = hxp.tile([1, H], f32, tag="hx", name="pz")
            for blk in range(FC1_NBLK):
                if blk + FC1_BUFS < FC1_NBLK:
                    issue_fc1(blk + FC1_BUFS)
                w = fc1_tiles[blk]
                for ch in range(NCH):
                    n = blk * NCH + ch
                    nc.tensor.matmul(
                        pz[:],
                        lhsT=h3q[:, :, n : n + 1],
                        rhs=w[:, ch],
                        start=(n == 0),
                        stop=(n == SHARD - 1),
                        perf_mode=DR,
                    )
            zsb = spool.tile([1, H], f32, tag="zsb")
            nc.vector.tensor_copy(zsb[:], pz[:])

            # AllGather per-core z partials, then sum over ranks on the PE
            # with a K=8 ones-matmul -- which also transposes z into the
            # [128, 1] column layout fc2 needs.
            agzin = dram.tile([1, H], f32, tag="agzin")
            nc.gpsimd.dma_start(agzin[:], zsb[:])
            agzout = dram.tile(
                [NCORES, 1, H], f32, tag="agzout", addr_space="Shared"
            )
            nc.gpsimd.collective_compute(
                "AllGather",
                ALU.bypass,
                replica_groups=rg,
                ins=[agzin.opt()],
                outs=[agzout.opt()],
            )
            zparts = spool.tile([NCORES, H], f32, tag="zparts")
            nc.sync.dma_start(zparts[:], agzout[:, 0, :])

            po = hxp.tile([C, 1], f32, tag="hx", name="po")
            for k in range(2):
                poz = aggp.tile([128, 1], f32, tag="agg", name=f"poz{k}")
                nc.tensor.matmul(
                    poz[:],
                    lhsT=zparts[:, k * 128 : (k + 1) * 128],
                    rhs=ones8[:],
                    start=True,
                    stop=True,
                )
                zr = spool.tile([128, 1], bf16, tag=f"zr{k}")
                nc.scalar.activation(
                    zr[:],
                    poz[:],
                    AF.Relu,
                    bias=fc1b_sb[:, k : k + 1],
                    scale=1.0 / (FC1_SCALE * H_SCALE),
                )
                nc.tensor.matmul(
                    po[:],
                    lhsT=fc2t_sb[:, k * C : (k + 1) * C],
                    rhs=zr[:],
                    start=(k == 0),
                    stop=(k == 1),
                )
            osb = spool.tile([C, 1], f32, tag="osb")
            nc.scalar.activation(osb[:], po[:], AF.Sigmoid, bias=fc2b_sb[:, 0:1])
            nc.gpsimd.dma_start(OUT[:], osb[:])

    nc.compile()
    _split_drain_waits(nc)
    return nc


def _prep_shared(feat, W0, b0, W1, b1, W2, b2, fc1_b, fc2_w, fc2_b):
    """Host layout prep for the tensors every core receives identically."""
    # g0[p, ch, i, f] = feat[ch*256 + i*128 + p, f] * H_SCALE
    g0 = np.ascontiguousarray(
        (feat * H_SCALE)
        .reshape(NCH, 2, 128, F_IN)
        .transpose(2, 0, 1, 3)
    ).astype(FP8)

    # wt[p, (l*NE+e)*2+fh, j] = W_l[e][j, fh*128+p]
    wt = np.empty((128, 3 * NE * 2, H), dtype=BF16)
    for li, W in enumerate((W0, W1, W2)):
        for e in range(NE):
            wte = np.ascontiguousarray(W[e].T).astype(BF16)  # [F, H]
            wt[:, (li * NE + e) * 2 + 0, :] = wte[:128]
            wt[:, (li * NE + e) * 2 + 1, :] = wte[128:]

    fc1b = np.ascontiguousarray(fc1_b.reshape(2, 128).T).astype(np.float32)
    fc2t = np.ascontiguousarray(
        fc2_w.T.reshape(2, 128, C).transpose(1, 0, 2).reshape(128, 2 * C)
    ).astype(BF16)
    fc2b = fc2_b.reshape(C, 1).astype(np.float32)
    return g0, wt, fc1b, fc2t, fc2b


def _prep_graph(edges):
    """Per-(etype, core) fp8 adjacency + exact drain corrections.

    A entry stored = cnt * q_d with q_d = fp8(A_SCALE/deg_d); corr so that
    q_d * corr_d == 1/deg_d exactly (in f32).
    """
    atp = np.empty((NCORES, NE, NCHP, 128, 2, 2, SHARD), dtype=FP8)
    corr = np.empty((NCORES, 128, NE, SHARD), dtype=np.float32)
    for e in range(NE):
        src = np.asarray(edges[e, 0], dtype=np.int64)
        dst = np.asarray(edges[e, 1], dtype=np.int64)
        deg = np.bincount(dst, minlength=N_OBJ).astype(np.float64)
        q = (A_SCALE / np.maximum(deg, 1.0)).astype(FP8).astype(np.float32)
        cnt = (
            np.bincount(src * N_OBJ + dst, minlength=N_OBJ * N_OBJ)
            .reshape(N_OBJ, N_OBJ)
            .astype(np.float32)
        )
        a_store = (cnt * q[None, :]).astype(FP8)  # [src, dst]
        corr_e = np.where(
            deg > 0, 1.0 / (np.maximum(deg, 1.0) * q.astype(np.float64)), 0.0
        ).astype(np.float32)
        for c in range(NCORES):
            sl = a_store[:, c * SHARD : (c + 1) * SHARD]  # [4096, 512]
            # [src, d] -> [chp, sub, i, p, d] -> [chp, p, sub, i, d]
            atp[c, e] = (
                sl.reshape(NCHP, 2, 2, 128, SHARD).transpose(0, 3, 1, 2, 4)
            )
            corr[c, :, e, :] = corr_e[c * SHARD : (c + 1) * SHARD][None, :]
    return atp, corr


def _prep_bias(edges, b0, b1, b2):
    """hbias[l][dst, j] = sum_e ind_e[dst] * b_l[e][j] (H_SCALE domain)."""
    bs = np.stack([np.asarray(b0), np.asarray(b1), np.asarray(b2)])
    if not np.any(bs):
        return None
    hb = np.zeros((3, N_OBJ, H), dtype=np.float64)
    for e in range(NE):
        dst = np.asarray(edges[e, 1], dtype=np.int64)
        ind = (np.bincount(dst, minlength=N_OBJ) > 0).astype(np.float64)
        for li in range(3):
            hb[li] += ind[:, None] * bs[li, e][None, :]
    hb *= H_SCALE
    hbn = np.empty((NCORES, 128, 2, 4, H), dtype=BF16)
    hbt = np.empty((NCORES, 128, 2, SHARD), dtype=BF16)
    for c in range(NCORES):
        own = hb[:, c * SHARD : (c + 1) * SHARD, :]  # [3, 512, H]
        for li in range(2):
            hbn[c, :, li] = own[li].reshape(4, 128, H).transpose(1, 0, 2)
        hbt[c] = own[2].T.reshape(2, 128, SHARD).transpose(1, 0, 2)
    return hbn, hbt


def _prep_fc1(fc1_w):
    """Per-core column slice of fc1_w: [blk, p, ch, i, o] fp8, DoubleRow."""
    out = []
    ksl = SHARD * H  # 131072 flat positions per core
    for c in range(NCORES):
        sl = np.ascontiguousarray(fc1_w[:, c * ksl : (c + 1) * ksl].T)
        packed = np.ascontiguousarray(
            (sl * FC1_SCALE)
            .reshape(FC1_NBLK, NCH, 2, 128, H)
            .transpose(0, 3, 1, 2, 4)
        ).astype(FP8)
        out.append(packed)
    return out


def _make_in_maps(inputs):
    feat = np.asarray(inputs["feat"], dtype=np.float32)
    edges = np.asarray(inputs["edges"])
    g0, wt, fc1b, fc2t, fc2b = _prep_shared(
        feat,
        np.asarray(inputs["W0"]), np.asarray(inputs["b0"]),
        np.asarray(inputs["W1"]), np.asarray(inputs["b1"]),
        np.asarray(inputs["W2"]), np.asarray(inputs["b2"]),
        np.asarray(inputs["fc1_b"]), np.asarray(inputs["fc2_w"]),
        np.asarray(inputs["fc2_b"]),
    )
    atp, corr = _prep_graph(edges)
    fc1t = _prep_fc1(np.asarray(inputs["fc1_w"]))
    hbias = _prep_bias(edges, inputs["b0"], inputs["b1"], inputs["b2"])
    maps = []
    for c in range(NCORES):
        m = {
            "g0": g0, "atp": atp[c], "corr": corr[c], "wt": wt,
            "fc1t": fc1t[c], "fc1b": fc1b, "fc2t": fc2t, "fc2b": fc2b,
        }
        if hbias is not None:
            m["hbn"] = hbias[0][c]
            m["hbt"] = hbias[1][c]
        maps.append(m)
    return maps, hbias is not None


def kernel(feat, edges, W0, b0, W1, b1, W2, b2, fc1_w, fc1_b, fc2_w, fc2_b):
    from concourse.bass_utils import run_bass_kernel_spmd

    in_maps, has_bias = _make_in_maps(
        dict(
            feat=feat, edges=edges, W0=W0, b0=b0, W1=W1, b1=b1, W2=W2, b2=b2,
            fc1_w=fc1_w, fc1_b=fc1_b, fc2_w=fc2_w, fc2_b=fc2_b,
        )
    )
    key = ("nc", has_bias)
    if key not in _BASS_CACHE:
        _BASS_CACHE[key] = _build_bass(has_bias=has_bias)
    nc = _BASS_CACHE[key]

    res = run_bass_kernel_spmd(nc, in_maps, core_ids=list(range(NCORES)))
    out = np.asarray(res.results[0]["out"]).reshape(C)
    return out.astype(np.float32)


def run_profiled(inputs, trace_cores=None):
    """Test-only: run with NTFF tracing; returns BassKernelResults."""
    from concourse import bass_utils
    from concourse.bass_utils import run_bass_kernel_spmd

    bass_utils.upload_artifacts = lambda tmpdir: f"local://{tmpdir}"
    in_maps, has_bias = _make_in_maps(inputs)
    key = ("nc", has_bias)
    if key not in _BASS_CACHE:
        _BASS_CACHE[key] = _build_bass(has_bias=has_bias)
    nc = _BASS_CACHE[key]
    tmpdir = "/tmp/gcn_profile"
    import shutil, os
    shutil.rmtree(tmpdir, ignore_errors=True)
    os.makedirs(tmpdir, exist_ok=True)
    return run_bass_kernel_spmd(
        nc,
        in_maps,
        core_ids=list(range(NCORES)),
        trace=True,
        tmpdir=tmpdir,
        trace_cores=trace_cores,
    )
